# revision 5
# baseline (speedup 1.0000x reference)
"""CenterLoss Trainium2 kernel.

Reference computes, for x[B,D], labels[B], centers[C,D]:
    distmat[b,c] = ||x_b||^2 + ||c_c||^2 - 2<x_b, c_c>
    dist = where(labels[b]==c, distmat, 0)
    loss = clip(dist, 1e-12, 1e12).sum() / B

Only one entry per row survives the mask: d_b = ||x_b - centers[labels_b]||^2.
The other C-1 zeros per row are clamped to 1e-12, contributing the constant
B*(C-1)*1e-12 to the sum.  So:

    loss = ( sum_b clip(d_b, 1e-12, 1e12) ) / B  +  (C-1)*1e-12

No [B,C] distmat needed: gather centers[labels] (indirect DMA), squared
distance per row, clip, reduce.  Data-parallel over batch across 8 cores;
centers stay in HBM and only the labeled rows are read (indirect gather).

Per-core layout: row r of the 1024-row shard lives at partition p = r//8,
free slot j = r%8, so the x load and label load are contiguous DMAs and the
gather writes [128, 8, 128].
"""

import numpy as np

B, C, D = 8192, 10000, 128
N_CORES = 8
RPC = B // N_CORES  # rows per core
P = 128
J = RPC // P  # free slots per partition

CLIP_LO = 1e-12
CLIP_HI = 1e12
MASK_CONST = (C - 1) * CLIP_LO  # clamped masked-out zeros, after /B

_cache = {}


def _build(dbg=False):
    import concourse.bacc as bacc
    import concourse.bass as bass
    import concourse.mybir as mybir
    import concourse.tile as tile

    f32 = mybir.dt.float32
    i32 = mybir.dt.int32

    nc = bacc.Bacc("TRN2", target_bir_lowering=False, debug=False)

    x_d = nc.dram_tensor("x", [RPC, D], f32, kind="ExternalInput")
    lab_d = nc.dram_tensor("labels", [P, J], i32, kind="ExternalInput")
    cen_d = nc.dram_tensor("centers", [C, D], f32, kind="ExternalInput")
    out_d = nc.dram_tensor("out", [1, 1], f32, kind="ExternalOutput")
    if dbg:
        dbg_ct = nc.dram_tensor("dbg_ct", [P, J, D], f32, kind="ExternalOutput")
        dbg_xt = nc.dram_tensor("dbg_xt", [P, J, D], f32, kind="ExternalOutput")
        dbg_dsum = nc.dram_tensor("dbg_dsum", [P, J], f32, kind="ExternalOutput")
        dbg_dtot = nc.dram_tensor("dbg_dtot", [P, 1], f32, kind="ExternalOutput")

    N_CHUNK = 2  # split gather/compute for DMA/compute overlap
    JC = J // N_CHUNK

    with tile.TileContext(nc) as tc:
        with (
            tc.tile_pool(name="sbuf", bufs=1) as pool,
            tc.tile_pool(name="psum", bufs=1, space="PSUM") as psum_pool,
        ):
            xt = pool.tile([P, J, D], f32)
            ct = pool.tile([P, J, D], f32)
            diff = pool.tile([P, J, D], f32)
            sq = pool.tile([P, J, D], f32)
            it = pool.tile([P, J], i32)
            dsum = pool.tile([P, J], f32)
            dclip = pool.tile([P, J], f32)
            dtot = pool.tile([P, 1], f32)
            ones = pool.tile([P, 1], f32)
            res = pool.tile([1, 1], f32)

            nc.sync.dma_start(out=it[:], in_=lab_d[:, :])
            nc.vector.memset(ones[:], 1.0)

            x_ap = x_d[:, :].rearrange("(p j) d -> p j d", p=P)
            for h in range(N_CHUNK):
                js = slice(h * JC, (h + 1) * JC)
                nc.sync.dma_start(out=xt[:, js, :], in_=x_ap[:, js, :])
                for j in range(h * JC, (h + 1) * JC):
                    nc.gpsimd.indirect_dma_start(
                        out=ct[:, j, :],
                        out_offset=None,
                        in_=cen_d[:, :],
                        in_offset=bass.IndirectOffsetOnAxis(ap=it[:, j : j + 1], axis=0),
                    )
                nc.vector.tensor_tensor(
                    out=diff[:, js, :],
                    in0=xt[:, js, :],
                    in1=ct[:, js, :],
                    op=mybir.AluOpType.subtract,
                )
                for j in range(h * JC, (h + 1) * JC):
                    nc.scalar.activation(
                        out=sq[:, j, :],
                        in_=diff[:, j, :],
                        func=mybir.ActivationFunctionType.Square,
                        accum_out=dsum[:, j : j + 1],
                    )

            # clip each per-row distance, then sum the J slots per partition
            nc.vector.tensor_scalar(
                out=dclip[:],
                in0=dsum[:],
                scalar1=CLIP_LO,
                scalar2=CLIP_HI,
                op0=mybir.AluOpType.max,
                op1=mybir.AluOpType.min,
            )
            nc.vector.tensor_reduce(
                out=dtot[:],
                in_=dclip[:],
                axis=mybir.AxisListType.X,
                op=mybir.AluOpType.add,
            )
            # cross-partition sum via PE: [1,1] = dtot[128,1].T @ ones[128,1]
            pt = psum_pool.tile([1, 1], f32)
            nc.tensor.matmul(out=pt[:], lhsT=dtot[:], rhs=ones[:], start=True, stop=True)
            nc.scalar.activation(
                out=res[:],
                in_=pt[:],
                func=mybir.ActivationFunctionType.Copy,
                scale=1.0 / B,
            )
            nc.sync.dma_start(out=out_d[:, :], in_=res[:])
            if dbg:
                nc.sync.dma_start(out=dbg_ct[:, :, :], in_=ct[:])
                nc.sync.dma_start(out=dbg_xt[:, :, :], in_=xt[:])
                nc.sync.dma_start(out=dbg_dsum[:, :], in_=dsum[:])
                nc.sync.dma_start(out=dbg_dtot[:, :], in_=dtot[:])

    nc.compile()
    return nc


def _get_nc():
    if "nc" not in _cache:
        _cache["nc"] = _build()
    return _cache["nc"]


def _make_in_maps(x, labels, centers):
    x = np.ascontiguousarray(np.asarray(x, dtype=np.float32))
    labels = np.asarray(labels).astype(np.int32)
    centers = np.ascontiguousarray(np.asarray(centers, dtype=np.float32))
    in_maps = []
    for i in range(N_CORES):
        sl = slice(i * RPC, (i + 1) * RPC)
        in_maps.append(
            {
                "x": x[sl],
                "labels": np.ascontiguousarray(labels[sl].reshape(P, J)),
                "centers": centers,
            }
        )
    return in_maps


def _run(in_maps, trace=False, **kwargs):
    from concourse.bass_utils import run_bass_kernel_spmd

    nc = _get_nc()
    return run_bass_kernel_spmd(
        nc, in_maps, core_ids=list(range(N_CORES)), trace=trace, **kwargs
    )


def kernel(x, labels, centers):
    res = _run(_make_in_maps(x, labels, centers))
    total = np.float32(0.0)
    for r in res.results:
        total += np.float32(r["out"].reshape(()))
    return np.asarray(total + np.float32(MASK_CONST), dtype=np.float32)
